# revision 25
# baseline (speedup 1.0000x reference)
"""Trainium2 Bass kernel for nn_CenterContrastiveLoss.

Problem: loss = label-smoothed CE over [pos, top-50 negs] of f @ centers.T
  f: [2048, 256] f32, centers: [65536, 256] f32, label: [2048] int.

Strategy (8 NeuronCores, tensor-parallel over C=65536):
  - fp8(e4m3) DoubleRow matmuls: K=256 contracted in a single pass per
    [128 x 512] PSUM slice (2 fp8 weights per PE cell, 2 MACs/cycle).
    Loop rt(16 row tiles) x q(4 col chunks of 2048) x h(2 halves):
    PSUM tiles [128, 1024] x 4 bufs for a deep fill/evict pipeline.
  - PSUM eviction split across engines to balance time:
      q0, q1 (and q2 when rt in ACT2_RT) -> ScalarE full-width f32->f16
            copy into a per-rt staging strip, one ~1-1.5MB DMA per rt
            (exact per-column device values for ~55% of columns);
      rest -> VectorE grouped max (8 cols/bucket) -> f16, small DMA per rt.
  - DMAs batched large; outputs issue on the sync HWDGE ring (scalar queue
    runs only ACTIVATEs), inputs split across sync+gpsimd rings.
  - Host: per row, select top-J candidates from device values (columns in
    the full region, buckets in the max region), expand buckets to their 8
    columns, recompute those few columns' scores exactly in f64 from the
    original f32 inputs, then evaluate the reference loss formula exactly
    on [pos, top-50]. This removes the fp8 noise from the final loss.
"""

import numpy as np
import ml_dtypes

B, C, D = 2048, 65536, 256
NCORES = 8
CSH = C // NCORES          # 8192
RT = B // 128              # 16
NQ = 4                     # column chunks per core
QW = CSH // NQ             # 2048
HW = QW // 2               # 1024 (psum tile width)
GR = 8                     # DVE reduce group width
NGB = QW // GR             # 256 buckets per reduced chunk
ACT2_RT = (0, 8)           # row-tiles whose q2 chunk goes to ScalarE
NFULL = 32 + len(ACT2_RT)  # 35 ACT-evicted (full f16) 2048-wide chunks
TOPJ = 96                  # host candidate selection width

_prog = None


def _is_a(rt):
    return rt in ACT2_RT


def _s0(rt):
    # first out_full slot of row-tile rt (slots within rt: q0, q1[, q2])
    return 2 * rt + sum(1 for r in ACT2_RT if r < rt)


def _build_program():
    import concourse.mybir as mybir
    from concourse import bacc
    from concourse.tile import TileContext
    from contextlib import ExitStack

    fp8 = mybir.dt.float8e4
    f16 = mybir.dt.float16
    f32 = mybir.dt.float32
    DR = mybir.MatmulPerfMode.DoubleRow

    nc = bacc.Bacc("TRN2")
    fT_d = nc.declare_dram_parameter("fT", [128, 2, B], fp8, isOutput=False)
    cT_d = nc.declare_dram_parameter("cT", [128, 2, CSH], fp8, isOutput=False)
    full_d = nc.declare_dram_parameter("out_full", [NFULL, 128, QW], f16,
                                       isOutput=True)
    red_d = nc.declare_dram_parameter("out_red", [RT, 128, 2 * NGB], f16,
                                      isOutput=True)

    with TileContext(nc) as tc, ExitStack() as ctx:
        const = ctx.enter_context(tc.tile_pool(name="const", bufs=1))
        psumA = ctx.enter_context(tc.tile_pool(name="psumA", bufs=2,
                                               space="PSUM"))
        psumD = ctx.enter_context(tc.tile_pool(name="psumD", bufs=2,
                                               space="PSUM"))
        fullst = ctx.enter_context(tc.tile_pool(name="fullst", bufs=4))
        redst = ctx.enter_context(tc.tile_pool(name="redst", bufs=2))

        fT_t = const.tile([128, 2 * B], fp8, tag="fT", name="fT")
        cT_t = const.tile([128, 2 * CSH], fp8, tag="cT", name="cT")
        fT_v = fT_t.rearrange("p (two b) -> p two b", two=2)
        cT_v = cT_t.rearrange("p (two w) -> p two w", two=2)

        # batched input DMAs, split across the sync and gpsimd rings;
        # ordered to match first-round consumption (q order 0, 2, 1, 3)
        nc.sync.dma_start(out=cT_v[:, :, 0:HW], in_=cT_d[:, :, 0:HW])
        nc.gpsimd.dma_start(out=fT_v[:, :, 0:128], in_=fT_d[:, :, 0:128])
        nc.sync.dma_start(out=cT_v[:, :, HW:QW], in_=cT_d[:, :, HW:QW])
        nc.gpsimd.dma_start(out=cT_v[:, :, 3 * QW:4 * QW],
                            in_=cT_d[:, :, 3 * QW:4 * QW])
        nc.sync.dma_start(out=cT_v[:, :, QW:2 * QW],
                          in_=cT_d[:, :, QW:2 * QW])
        nc.gpsimd.dma_start(out=cT_v[:, :, 2 * QW:3 * QW],
                            in_=cT_d[:, :, 2 * QW:3 * QW])
        nc.gpsimd.dma_start(out=fT_v[:, :, 128:B], in_=fT_d[:, :, 128:B])

        for rt in range(RT):
            lhsT = fT_v[:, :, rt * 128:(rt + 1) * 128]
            s0 = _s0(rt)
            rs = redst.tile([128, 2 * NGB], f16, tag="rs", name="rs")
            qorder = (0, 3, 1, 2) if rt == 0 else (0, 2, 1, 3)
            for q in qorder:
                act = q < 2 or (q == 2 and _is_a(rt))
                if act:
                    fs = fullst.tile([128, QW], f16, tag="fs", name="fs")
                for h in range(2):
                    pool = psumA if act else psumD
                    pt = pool.tile([128, HW], f32, tag="pt", name="pt")
                    base = q * QW + h * HW
                    for c in range(2):
                        nc.tensor.matmul(
                            pt[:, c * 512:(c + 1) * 512],
                            lhsT,
                            cT_v[:, :, base + c * 512:base + (c + 1) * 512],
                            start=True,
                            stop=True,
                            perf_mode=DR,
                        )
                    if act:
                        nc.scalar.copy(fs[:, h * HW:(h + 1) * HW], pt[:])
                    else:
                        off = (0 if q == 3 else NGB) + h * (NGB // 2)
                        nc.vector.tensor_reduce(
                            out=rs[:, off:off + NGB // 2],
                            in_=pt[:].rearrange("p (g e) -> p g e", e=GR),
                            axis=mybir.AxisListType.X,
                            op=mybir.AluOpType.max,
                        )
                if act:
                    nc.sync.dma_start(out=full_d[s0 + min(q, 2)], in_=fs[:])
            rw = NGB if _is_a(rt) else 2 * NGB
            nc.sync.dma_start(out=red_d[rt, :, 0:rw], in_=rs[:, 0:rw])

    nc.finalize()
    return nc


def _get_program():
    global _prog
    if _prog is None:
        _prog = _build_program()
    return _prog


def run_device(in_maps, trace=False, **kw):
    from concourse.bass_utils import run_bass_kernel_spmd

    nc = _get_program()
    return run_bass_kernel_spmd(nc, in_maps, core_ids=list(range(NCORES)),
                                trace=trace, **kw)


def make_in_maps(f, centers, label):
    fp8 = ml_dtypes.float8_e4m3fn
    f8 = f.astype(fp8)
    c8 = centers.astype(fp8)
    # [b, d] -> [partition, k-half, col]  (d = 128*k + p)
    fT = np.ascontiguousarray(f8.T.reshape(2, 128, B).transpose(1, 0, 2))
    in_maps = []
    for core in range(NCORES):
        cT = np.ascontiguousarray(
            c8[core * CSH:(core + 1) * CSH].T.reshape(2, 128, CSH)
            .transpose(1, 0, 2))
        in_maps.append({"fT": fT, "cT": cT})
    return in_maps


def postprocess(results, f, centers, label):
    f64 = np.float64
    rows = np.arange(B)
    rt_of_row = rows // 128
    classA = np.isin(rt_of_row, ACT2_RT)   # rows whose q2 chunk is full
    WA = 4096 + 2048 + NGB                 # per-core candidate width, class A
    WB = 4096 + NGB + NGB                  # per-core candidate width, class B
    NA = len(ACT2_RT)
    NB = RT - NA

    candA = np.empty((NA * 128, NCORES * WA), dtype=np.float32)
    candB = np.empty((NB * 128, NCORES * WB), dtype=np.float32)
    rowsA = rows[classA]
    rowsB = rows[~classA]

    for m, res in enumerate(results):
        fullv = np.asarray(res["out_full"], dtype=np.float32)  # [35,128,QW]
        redv = np.asarray(res["out_red"], dtype=np.float32)    # [16,128,512]
        s0s = [_s0(rt) for rt in range(RT)]
        q01 = np.stack([fullv[[s for s in s0s]],
                        fullv[[s + 1 for s in s0s]]], axis=1)
        full01 = q01.transpose(0, 2, 1, 3).reshape(B, 2 * QW)  # [B, 4096]
        q2full = fullv[[s0s[rt] + 2 for rt in ACT2_RT]] \
            .reshape(NA * 128, QW)                             # rowsA order
        q3red = redv[:, :, 0:NGB].reshape(B, NGB)
        q2red = redv[[rt for rt in range(RT) if not _is_a(rt)], :, NGB:2 * NGB] \
            .reshape(NB * 128, NGB)                            # rowsB order

        candA[:, m * WA:(m + 1) * WA] = np.concatenate(
            [full01[rowsA], q2full, q3red[rowsA]], axis=1)
        candB[:, m * WB:(m + 1) * WB] = np.concatenate(
            [full01[rowsB], q2red, q3red[rowsB]], axis=1)

    def decode(sel_idx, wpc, is_a):
        """Map per-class candidate index -> (up to 8) global column ids."""
        m = sel_idx // wpc
        r = sel_idx % wpc
        base = m * CSH
        nrow, J = sel_idx.shape
        cols = np.full((nrow, J, GR), -1, dtype=np.int64)
        if is_a:
            isfull = r < 6144
            bstart = 6144 + (r - 6144) * GR
        else:
            isfull = r < 4096
            bstart = np.where(r < 4352, 4096 + (r - 4096) * GR,
                              6144 + (r - 4352) * GR)
        cols[:, :, 0] = np.where(isfull, base + r, -1)
        bcols = (base + bstart)[:, :, None] + np.arange(GR)[None, None, :]
        cols = np.where(isfull[:, :, None], cols, bcols)
        return cols.reshape(nrow, J * GR)

    selA = np.argpartition(-candA, TOPJ - 1, axis=1)[:, :TOPJ]
    selB = np.argpartition(-candB, TOPJ - 1, axis=1)[:, :TOPJ]
    colsA = decode(selA, WA, True)
    colsB = decode(selB, WB, False)

    cols = np.empty((B, TOPJ * GR), dtype=np.int64)
    cols[rowsA] = colsA
    cols[rowsB] = colsB

    # exact recompute of the selected columns in f64
    fd = f.astype(f64)
    valid = cols >= 0
    safe_cols = np.where(valid, cols, 0)
    exact = np.empty(cols.shape, dtype=f64)
    chunk = 128
    for i in range(0, B, chunk):
        cc = centers[safe_cols[i:i + chunk]].astype(f64)   # [ch, J*GR, D]
        exact[i:i + chunk] = np.einsum("bjd,bd->bj", cc, fd[i:i + chunk])
    exact[~valid] = -np.inf
    exact[cols == label[:, None]] = -np.inf

    top50 = -np.partition(-exact, 49, axis=1)[:, :50]
    pos = np.einsum("bd,bd->b", centers[label].astype(f64), fd)

    preds = np.concatenate([pos[:, None], top50], axis=1)
    mx = preds.max(axis=1, keepdims=True)
    lse = mx[:, 0] + np.log(np.exp(preds - mx).sum(axis=1))
    S1 = top50.sum(axis=1)
    loss = np.mean(0.9102 * lse - 0.9002 * pos - 0.0002 * S1)
    return np.array(loss, dtype=np.float32)


def kernel(f, centers, label):
    f = np.asarray(f, dtype=np.float32)
    centers = np.asarray(centers, dtype=np.float32)
    label = np.asarray(label).astype(np.int64)
    in_maps = make_in_maps(f, centers, label)
    try:
        res = run_device(in_maps)
    except Exception:
        # transient runtime flakes (e.g. NRT_EXEC_UNIT_UNRECOVERABLE) have
        # been observed to succeed on immediate retry
        res = run_device(in_maps)
    return postprocess(res.results, f, centers, label)


# revision 26
# speedup vs baseline: 1.0271x; 1.0271x over previous
"""Trainium2 Bass kernel for nn_CenterContrastiveLoss.

Problem: loss = label-smoothed CE over [pos, top-50 negs] of f @ centers.T
  f: [2048, 256] f32, centers: [65536, 256] f32, label: [2048] int.

Strategy (8 NeuronCores, tensor-parallel over C=65536):
  - fp8(e4m3) DoubleRow matmuls: K=256 contracted in a single pass per
    [128 x 512] PSUM slice (2 fp8 weights per PE cell, 2 MACs/cycle).
    Loop rt(16 row tiles) x q(4 col chunks of 2048) x h(2 halves):
    PSUM tiles [128, 1024] x 4 bufs for a deep fill/evict pipeline.
  - PSUM eviction split across engines to balance time:
      q0, q1 (and q2 when rt in ACT2_RT) -> ScalarE full-width f32->f16
            copy into a per-rt staging strip, one ~1-1.5MB DMA per rt
            (exact per-column device values for ~55% of columns);
      rest -> VectorE grouped max (8 cols/bucket) -> f16, small DMA per rt.
  - DMAs batched large; outputs issue on the sync HWDGE ring (scalar queue
    runs only ACTIVATEs), inputs split across sync+gpsimd rings.
  - Host: per row, select top-J candidates from device values (columns in
    the full region, buckets in the max region), expand buckets to their 8
    columns, recompute those few columns' scores exactly in f64 from the
    original f32 inputs, then evaluate the reference loss formula exactly
    on [pos, top-50]. This removes the fp8 noise from the final loss.
"""

import numpy as np
import ml_dtypes

B, C, D = 2048, 65536, 256
NCORES = 8
CSH = C // NCORES          # 8192
RT = B // 128              # 16
NQ = 4                     # column chunks per core
QW = CSH // NQ             # 2048
HW = QW // 2               # 1024 (psum tile width)
GR = 8                     # DVE reduce group width
NGB = QW // GR             # 256 buckets per reduced chunk
ACT2_RT = (4, 8)           # row-tiles whose q2 chunk goes to ScalarE
NFULL = 32 + len(ACT2_RT)  # 35 ACT-evicted (full f16) 2048-wide chunks
TOPJ = 96                  # host candidate selection width

_prog = None


def _is_a(rt):
    return rt in ACT2_RT


def _s0(rt):
    # first out_full slot of row-tile rt (slots within rt: q0, q1[, q2])
    return 2 * rt + sum(1 for r in ACT2_RT if r < rt)


def _build_program():
    import concourse.mybir as mybir
    from concourse import bacc
    from concourse.tile import TileContext
    from contextlib import ExitStack

    fp8 = mybir.dt.float8e4
    f16 = mybir.dt.float16
    f32 = mybir.dt.float32
    DR = mybir.MatmulPerfMode.DoubleRow

    nc = bacc.Bacc("TRN2")
    fT_d = nc.declare_dram_parameter("fT", [128, 2, B], fp8, isOutput=False)
    cT_d = nc.declare_dram_parameter("cT", [128, 2, CSH], fp8, isOutput=False)
    full_d = nc.declare_dram_parameter("out_full", [NFULL, 128, QW], f16,
                                       isOutput=True)
    red_d = nc.declare_dram_parameter("out_red", [RT, 128, 2 * NGB], f16,
                                      isOutput=True)

    with TileContext(nc) as tc, ExitStack() as ctx:
        const = ctx.enter_context(tc.tile_pool(name="const", bufs=1))
        psumA = ctx.enter_context(tc.tile_pool(name="psumA", bufs=2,
                                               space="PSUM"))
        psumD = ctx.enter_context(tc.tile_pool(name="psumD", bufs=2,
                                               space="PSUM"))
        fullst = ctx.enter_context(tc.tile_pool(name="fullst", bufs=4))
        redst = ctx.enter_context(tc.tile_pool(name="redst", bufs=2))

        fT_t = const.tile([128, 2 * B], fp8, tag="fT", name="fT")
        cT_t = const.tile([128, 2 * CSH], fp8, tag="cT", name="cT")
        fT_v = fT_t.rearrange("p (two b) -> p two b", two=2)
        cT_v = cT_t.rearrange("p (two w) -> p two w", two=2)

        # batched input DMAs, split across the sync and gpsimd rings;
        # ordered to match first-round consumption (q order 0, 2, 1, 3)
        nc.sync.dma_start(out=cT_v[:, :, 0:HW], in_=cT_d[:, :, 0:HW])
        nc.gpsimd.dma_start(out=fT_v[:, :, 0:128], in_=fT_d[:, :, 0:128])
        nc.sync.dma_start(out=cT_v[:, :, HW:QW], in_=cT_d[:, :, HW:QW])
        nc.gpsimd.dma_start(out=cT_v[:, :, 3 * QW:4 * QW],
                            in_=cT_d[:, :, 3 * QW:4 * QW])
        nc.sync.dma_start(out=cT_v[:, :, QW:2 * QW],
                          in_=cT_d[:, :, QW:2 * QW])
        nc.gpsimd.dma_start(out=cT_v[:, :, 2 * QW:3 * QW],
                            in_=cT_d[:, :, 2 * QW:3 * QW])
        nc.gpsimd.dma_start(out=fT_v[:, :, 128:B], in_=fT_d[:, :, 128:B])

        for rt in range(RT):
            lhsT = fT_v[:, :, rt * 128:(rt + 1) * 128]
            s0 = _s0(rt)
            rs = redst.tile([128, 2 * NGB], f16, tag="rs", name="rs")
            qorder = (0, 3, 1, 2) if rt == 0 else (0, 2, 1, 3)
            for q in qorder:
                act = q < 2 or (q == 2 and _is_a(rt))
                if act:
                    fs = fullst.tile([128, QW], f16, tag="fs", name="fs")
                for h in range(2):
                    pool = psumA if act else psumD
                    pt = pool.tile([128, HW], f32, tag="pt", name="pt")
                    base = q * QW + h * HW
                    for c in range(2):
                        nc.tensor.matmul(
                            pt[:, c * 512:(c + 1) * 512],
                            lhsT,
                            cT_v[:, :, base + c * 512:base + (c + 1) * 512],
                            start=True,
                            stop=True,
                            perf_mode=DR,
                        )
                    if act:
                        nc.scalar.copy(fs[:, h * HW:(h + 1) * HW], pt[:])
                    else:
                        off = (0 if q == 3 else NGB) + h * (NGB // 2)
                        nc.vector.tensor_reduce(
                            out=rs[:, off:off + NGB // 2],
                            in_=pt[:].rearrange("p (g e) -> p g e", e=GR),
                            axis=mybir.AxisListType.X,
                            op=mybir.AluOpType.max,
                        )
                if act:
                    nc.sync.dma_start(out=full_d[s0 + min(q, 2)], in_=fs[:])
            rw = NGB if _is_a(rt) else 2 * NGB
            nc.sync.dma_start(out=red_d[rt, :, 0:rw], in_=rs[:, 0:rw])

    nc.finalize()
    return nc


def _get_program():
    global _prog
    if _prog is None:
        _prog = _build_program()
    return _prog


def run_device(in_maps, trace=False, **kw):
    from concourse.bass_utils import run_bass_kernel_spmd

    nc = _get_program()
    return run_bass_kernel_spmd(nc, in_maps, core_ids=list(range(NCORES)),
                                trace=trace, **kw)


def make_in_maps(f, centers, label):
    fp8 = ml_dtypes.float8_e4m3fn
    f8 = f.astype(fp8)
    c8 = centers.astype(fp8)
    # [b, d] -> [partition, k-half, col]  (d = 128*k + p)
    fT = np.ascontiguousarray(f8.T.reshape(2, 128, B).transpose(1, 0, 2))
    in_maps = []
    for core in range(NCORES):
        cT = np.ascontiguousarray(
            c8[core * CSH:(core + 1) * CSH].T.reshape(2, 128, CSH)
            .transpose(1, 0, 2))
        in_maps.append({"fT": fT, "cT": cT})
    return in_maps


def postprocess(results, f, centers, label):
    f64 = np.float64
    rows = np.arange(B)
    rt_of_row = rows // 128
    classA = np.isin(rt_of_row, ACT2_RT)   # rows whose q2 chunk is full
    WA = 4096 + 2048 + NGB                 # per-core candidate width, class A
    WB = 4096 + NGB + NGB                  # per-core candidate width, class B
    NA = len(ACT2_RT)
    NB = RT - NA

    candA = np.empty((NA * 128, NCORES * WA), dtype=np.float32)
    candB = np.empty((NB * 128, NCORES * WB), dtype=np.float32)
    rowsA = rows[classA]
    rowsB = rows[~classA]

    for m, res in enumerate(results):
        fullv = np.asarray(res["out_full"], dtype=np.float32)  # [35,128,QW]
        redv = np.asarray(res["out_red"], dtype=np.float32)    # [16,128,512]
        s0s = [_s0(rt) for rt in range(RT)]
        q01 = np.stack([fullv[[s for s in s0s]],
                        fullv[[s + 1 for s in s0s]]], axis=1)
        full01 = q01.transpose(0, 2, 1, 3).reshape(B, 2 * QW)  # [B, 4096]
        q2full = fullv[[s0s[rt] + 2 for rt in ACT2_RT]] \
            .reshape(NA * 128, QW)                             # rowsA order
        q3red = redv[:, :, 0:NGB].reshape(B, NGB)
        q2red = redv[[rt for rt in range(RT) if not _is_a(rt)], :, NGB:2 * NGB] \
            .reshape(NB * 128, NGB)                            # rowsB order

        candA[:, m * WA:(m + 1) * WA] = np.concatenate(
            [full01[rowsA], q2full, q3red[rowsA]], axis=1)
        candB[:, m * WB:(m + 1) * WB] = np.concatenate(
            [full01[rowsB], q2red, q3red[rowsB]], axis=1)

    def decode(sel_idx, wpc, is_a):
        """Map per-class candidate index -> (up to 8) global column ids."""
        m = sel_idx // wpc
        r = sel_idx % wpc
        base = m * CSH
        nrow, J = sel_idx.shape
        cols = np.full((nrow, J, GR), -1, dtype=np.int64)
        if is_a:
            isfull = r < 6144
            bstart = 6144 + (r - 6144) * GR
        else:
            isfull = r < 4096
            bstart = np.where(r < 4352, 4096 + (r - 4096) * GR,
                              6144 + (r - 4352) * GR)
        cols[:, :, 0] = np.where(isfull, base + r, -1)
        bcols = (base + bstart)[:, :, None] + np.arange(GR)[None, None, :]
        cols = np.where(isfull[:, :, None], cols, bcols)
        return cols.reshape(nrow, J * GR)

    selA = np.argpartition(-candA, TOPJ - 1, axis=1)[:, :TOPJ]
    selB = np.argpartition(-candB, TOPJ - 1, axis=1)[:, :TOPJ]
    colsA = decode(selA, WA, True)
    colsB = decode(selB, WB, False)

    cols = np.empty((B, TOPJ * GR), dtype=np.int64)
    cols[rowsA] = colsA
    cols[rowsB] = colsB

    # exact recompute of the selected columns in f64
    fd = f.astype(f64)
    valid = cols >= 0
    safe_cols = np.where(valid, cols, 0)
    exact = np.empty(cols.shape, dtype=f64)
    chunk = 128
    for i in range(0, B, chunk):
        cc = centers[safe_cols[i:i + chunk]].astype(f64)   # [ch, J*GR, D]
        exact[i:i + chunk] = np.einsum("bjd,bd->bj", cc, fd[i:i + chunk])
    exact[~valid] = -np.inf
    exact[cols == label[:, None]] = -np.inf

    top50 = -np.partition(-exact, 49, axis=1)[:, :50]
    pos = np.einsum("bd,bd->b", centers[label].astype(f64), fd)

    preds = np.concatenate([pos[:, None], top50], axis=1)
    mx = preds.max(axis=1, keepdims=True)
    lse = mx[:, 0] + np.log(np.exp(preds - mx).sum(axis=1))
    S1 = top50.sum(axis=1)
    loss = np.mean(0.9102 * lse - 0.9002 * pos - 0.0002 * S1)
    return np.array(loss, dtype=np.float32)


def kernel(f, centers, label):
    f = np.asarray(f, dtype=np.float32)
    centers = np.asarray(centers, dtype=np.float32)
    label = np.asarray(label).astype(np.int64)
    in_maps = make_in_maps(f, centers, label)
    try:
        res = run_device(in_maps)
    except Exception:
        # transient runtime flakes (e.g. NRT_EXEC_UNIT_UNRECOVERABLE) have
        # been observed to succeed on immediate retry
        res = run_device(in_maps)
    return postprocess(res.results, f, centers, label)
